# revision 19
# baseline (speedup 1.0000x reference)
"""GRU (CustomRNN) Trainium2 kernel.

Strategy: data-parallel over batch (256 -> 8 cores x 32). Per core:
  - x is pre-transposed on host to [4, 128, SEQ*32] (d_in on partitions,
    column = t*32 + b) so the x@W projections and h@U recurrences both run
    with the contraction dim on partitions without any on-device transposes.
  - Recurrent state h is kept in a packed-transposed layout [128, 128]
    (partition q, column 32k+b) representing hT[i=128k+q, b]; fp32 master
    plus a bf16 copy that feeds the matmuls.
  - Per T_BLK-step block: DMA x block (bf16), precompute A = x@W_{z,r,h} +
    bias with bf16 matmuls (N=512, fast weight load) into bf16 SBUF
    (contiguous per-step [128,128] slices), then the sequential GRU steps:
    per gate, A is injected into PSUM with a bf16 identity matmul
    (start=True) and 16 small bf16 matmuls (stationary U tile [128,128]
    with fast weight load, moving hT slice [128,32]) accumulate on top.
    Activations read PSUM directly. Update uses h' = (1-z)*h + z*htild
    with 1-z = sigmoid(-z_pre) computed on the scalar engine, so only
    z*htild and the final add sit on the critical path after tanh.
  - After all steps: relu(h) @ Wfc on device; gather + bias + batch-axis
    log_softmax on host (softmax is over the batch axis => cross-core).
"""

import numpy as np

import concourse.bass as bass
import concourse.mybir as mybir
import concourse.tile as tile
from concourse import bacc
from concourse.bass import ds
from concourse.bass_utils import run_bass_kernel_spmd

SEQ, BATCH, D_IN, D_HID, D_OUT = 2048, 256, 512, 512, 1000
NCORES = 8
BS = BATCH // NCORES          # 32 batch rows per core
KI = D_IN // 128              # 4 contraction chunks for x@W
KH = D_HID // 128             # 4 contraction chunks for h@U
T_BLK = 64                    # timesteps per block
F32 = mybir.dt.float32
F32R = mybir.dt.float32r
BF16 = mybir.dt.bfloat16
AF = mybir.ActivationFunctionType


def build_bass(seq=SEQ, t_blk=T_BLK):
    assert seq % t_blk == 0
    nblk = seq // t_blk
    CB = t_blk * BS           # x/A columns per block
    NCH = 512                 # psum free-dim chunk for the A matmuls
    nch = CB // NCH
    G = 3 * D_HID

    nc = bacc.Bacc(None, target_bir_lowering=False)

    x_d = nc.dram_tensor("xt", [KI, 128, seq * BS], BF16, kind="ExternalInput")
    w_d = nc.dram_tensor("w", [KI, 128, G], BF16, kind="ExternalInput")
    u_d = nc.dram_tensor("u", [KH, 128, G], BF16, kind="ExternalInput")
    b_d = nc.dram_tensor("bias", [128, 12], F32, kind="ExternalInput")
    i_d = nc.dram_tensor("ident", [128, 128], BF16, kind="ExternalInput")
    wfc_d = nc.dram_tensor("wfc", [KH, 128, D_OUT], F32, kind="ExternalInput")
    out_d = nc.dram_tensor("out", [BS, D_OUT], F32, kind="ExternalOutput")

    with tile.TileContext(nc) as tc:
        with (
            tc.tile_pool(name="const", bufs=1) as constp,
            tc.tile_pool(name="xb", bufs=1) as xpool,
            tc.tile_pool(name="ab", bufs=1) as apool,
            tc.tile_pool(name="st", bufs=2) as stp,
            tc.tile_pool(name="ps", bufs=2, space="PSUM") as psp,
            tc.tile_pool(name="psA", bufs=2, space="PSUM") as psa,
        ):
            u_sb = constp.tile([128, KH, G], BF16)
            w_sb = constp.tile([128, KI, G], BF16)
            b_sb = constp.tile([128, 12], F32)
            ident = constp.tile([128, 128], BF16)
            for k in range(KH):
                nc.sync.dma_start(u_sb[:, k, :], u_d[k])
            for k in range(KI):
                nc.sync.dma_start(w_sb[:, k, :], w_d[k])
            nc.sync.dma_start(b_sb[:], b_d[:])
            nc.sync.dma_start(ident[:], i_d[:])

            # ping/pong recurrent state, packed-T [128, 128] (col = 32k + b)
            h0 = constp.tile([128, KH * BS], F32)
            h1 = constp.tile([128, KH * BS], F32)
            h0b = constp.tile([128, KH * BS], BF16)
            h1b = constp.tile([128, KH * BS], BF16)
            nc.vector.memset(h0[:], 0.0)
            nc.vector.memset(h0b[:], 0.0)

            with tc.For_i(0, nblk, 1, hint_engines=(mybir.EngineType.PE,)) as ib:
                xblk = xpool.tile([128, KI, CB], BF16, tag="xblk")
                nc.sync.dma_start(
                    xblk[:],
                    x_d[:, :, ds(ib * CB, CB)].rearrange("k q c -> q k c"))

                # A[q, t, g, mj*32+b]: per-(t,g) slice is a contiguous
                # [128, 128] in the packed-T layout
                a_sb = apool.tile([128, t_blk, 3, KH * BS], BF16, tag="ablk")
                for g in range(3):
                    for mj in range(KH):
                        w_tile = w_sb[:, :, g * D_HID + mj * 128:
                                      g * D_HID + (mj + 1) * 128]
                        for ci in range(nch):
                            tch = NCH // BS      # timesteps per psum chunk
                            t0 = ci * tch
                            pa = psa.tile([128, NCH], F32, tag="pa")
                            for k in range(KI):
                                nc.tensor.matmul(
                                    pa[:],
                                    w_tile[:, k, :],
                                    xblk[:, k, ci * NCH:(ci + 1) * NCH],
                                    start=(k == 0),
                                    stop=(k == KI - 1),
                                )
                            a_out = a_sb[:, t0:t0 + tch, g,
                                         mj * BS:(mj + 1) * BS]
                            bias_ap = b_sb[:, g * 4 + mj:g * 4 + mj + 1]
                            if (g * KH + mj) % 2 == 0:
                                nc.vector.tensor_add(
                                    a_out,
                                    pa[:].rearrange("p (t b) -> p t b", b=BS),
                                    bias_ap[:, :, None]
                                    .to_broadcast((128, tch, BS)),
                                )
                            else:
                                nc.scalar.add(
                                    a_out,
                                    pa[:].rearrange("p (t b) -> p t b", b=BS),
                                    bias_ap,
                                )

                for t in range(t_blk):
                    hin = h0 if t % 2 == 0 else h1
                    hinb = h0b if t % 2 == 0 else h1b
                    hout = h1 if t % 2 == 0 else h0
                    houtb = h1b if t % 2 == 0 else h0b

                    pr = psp.tile([128, KH * BS], F32, tag="pr")
                    pz = psp.tile([128, KH * BS], F32, tag="pz")
                    ph = psp.tile([128, KH * BS], F32, tag="ph")

                    # inject A into psum (h-independent, scheduled early)
                    nc.tensor.matmul(pr[:], ident[:], a_sb[:, t, 1, :],
                                     start=True, stop=False,
                                     skip_group_check=True)
                    nc.tensor.matmul(pz[:], ident[:], a_sb[:, t, 0, :],
                                     start=True, stop=False,
                                     skip_group_check=True)

                    # r gate first (it gates the htild matmul)
                    for gate, ps in ((1, pr), (0, pz)):
                        off = gate * D_HID
                        for mj in range(KH):
                            for k in range(KH):
                                nc.tensor.matmul(
                                    ps[:, mj * BS:(mj + 1) * BS],
                                    u_sb[:, k, off + mj * 128:off + (mj + 1) * 128],
                                    hinb[:, k * BS:(k + 1) * BS],
                                    start=False,
                                    stop=(k == KH - 1),
                                    skip_group_check=True,
                                )

                    r_act = stp.tile([128, KH * BS], BF16, tag="r_act")
                    nc.scalar.activation(r_act[:], pr[:], AF.Sigmoid)
                    rh = stp.tile([128, KH * BS], BF16, tag="rh")
                    nc.vector.tensor_mul(rh[:], r_act[:], hinb[:])

                    nc.tensor.matmul(ph[:], ident[:], a_sb[:, t, 2, :],
                                     start=True, stop=False,
                                     skip_group_check=True)
                    off = 2 * D_HID
                    for mj in range(KH):
                        for k in range(KH):
                            nc.tensor.matmul(
                                ph[:, mj * BS:(mj + 1) * BS],
                                u_sb[:, k, off + mj * 128:off + (mj + 1) * 128],
                                rh[:, k * BS:(k + 1) * BS],
                                start=False,
                                stop=(k == KH - 1),
                                skip_group_check=True,
                            )

                    z_act = stp.tile([128, KH * BS], F32, tag="z_act")
                    nc.scalar.activation(z_act[:], pz[:], AF.Sigmoid)
                    zc_act = stp.tile([128, KH * BS], F32, tag="zc_act")
                    nc.scalar.activation(zc_act[:], pz[:], AF.Sigmoid,
                                         scale=-1.0)
                    # t1 = (1-z)*h, ready before tanh completes
                    t1 = stp.tile([128, KH * BS], F32, tag="t1")
                    nc.vector.tensor_mul(t1[:], zc_act[:], hin[:])

                    ht = stp.tile([128, KH * BS], F32, tag="ht")
                    nc.scalar.activation(ht[:], ph[:], AF.Tanh)
                    t2 = stp.tile([128, KH * BS], F32, tag="t2")
                    nc.vector.tensor_mul(t2[:], z_act[:], ht[:])
                    # bf16 state first: it feeds the next step's matmuls
                    nc.vector.tensor_add(houtb[:], t1[:], t2[:])
                    nc.vector.tensor_add(hout[:], t1[:], t2[:])

            # final state lands in h0 (t_blk even); fc head
            wfc_sb = constp.tile([128, KH, D_OUT], F32)
            for k in range(KH):
                nc.sync.dma_start(wfc_sb[:, k, :], wfc_d[k])
            hrelu = stp.tile([128, KH * BS], F32, tag="hrelu")
            nc.scalar.activation(hrelu[:], h0[:], AF.Relu)
            out_sb = stp.tile([BS, D_OUT], F32, tag="outsb")
            for ci in range(2):
                n0, nsz = ci * 500, 500
                po = psa.tile([128, NCH], F32, tag="pa")
                for k in range(KH):
                    nc.tensor.matmul(
                        po[:BS, :nsz],
                        hrelu[:, k * BS:(k + 1) * BS],
                        wfc_sb[:, k, n0:n0 + nsz],
                        start=(k == 0),
                        stop=(k == KH - 1),
                    )
                nc.vector.tensor_copy(out_sb[:, n0:n0 + nsz], po[:BS, :nsz])
            nc.sync.dma_start(out_d[:], out_sb[:])

    nc.finalize()
    return nc


def _prep_inputs(x, Wz, Uz, Wr, Ur, Wh, Uh, bz, buz, br, bur, bh, buh, Wfc):
    import ml_dtypes
    seq = x.shape[0]
    # x[t, 32c+b, 128k+q] -> xt[c][k, q, t*32+b]
    xr = x.reshape(seq, NCORES, BS, KI, 128).transpose(1, 3, 4, 0, 2)
    xt = np.ascontiguousarray(
        xr.astype(ml_dtypes.bfloat16)).reshape(NCORES, KI, 128, seq * BS)

    w_all = np.concatenate([Wz, Wr, Wh], axis=1)     # [512, 1536]
    u_all = np.concatenate([Uz, Ur, Uh], axis=1)
    w_dev = np.ascontiguousarray(
        w_all.reshape(KI, 128, 3 * D_HID)).astype(ml_dtypes.bfloat16)
    u_dev = np.ascontiguousarray(
        u_all.reshape(KH, 128, 3 * D_HID)).astype(ml_dtypes.bfloat16)
    b_all = np.stack([bz + buz, br + bur, bh + buh])  # [3, 512]
    b_dev = np.ascontiguousarray(
        b_all.reshape(3, 4, 128).transpose(2, 0, 1).reshape(128, 12))
    i_dev = np.eye(128, dtype=np.float32).astype(ml_dtypes.bfloat16)
    wfc_dev = np.ascontiguousarray(Wfc.reshape(KH, 128, D_OUT))
    return xt, w_dev, u_dev, b_dev, i_dev, wfc_dev


def make_in_maps(inputs, seq=SEQ):
    f = lambda k: np.ascontiguousarray(np.asarray(inputs[k], dtype=np.float32))
    x = f("x")[:seq]
    xt, w_dev, u_dev, b_dev, i_dev, wfc_dev = _prep_inputs(
        x, f("Wz"), f("Uz"), f("Wr"), f("Ur"), f("Wh"), f("Uh"),
        f("bz"), f("buz"), f("br"), f("bur"), f("bh"), f("buh"), f("Wfc"))
    return [
        {"xt": xt[c], "w": w_dev, "u": u_dev, "bias": b_dev, "ident": i_dev,
         "wfc": wfc_dev}
        for c in range(NCORES)
    ]


def run_gru(inputs, seq=SEQ, t_blk=T_BLK, trace=False):
    in_maps = make_in_maps(inputs, seq=seq)
    nc = build_bass(seq=seq, t_blk=t_blk)
    res = run_bass_kernel_spmd(nc, in_maps, core_ids=list(range(NCORES)),
                               trace=trace)
    logits = np.concatenate([res.results[c]["out"] for c in range(NCORES)], 0)
    logits = logits + np.asarray(inputs["bfc"], np.float32)[None, :]
    m = logits.max(axis=0, keepdims=True)
    lse = m + np.log(np.exp(logits - m).sum(axis=0, keepdims=True))
    out = (logits - lse)[None]
    return out.astype(np.float32), res


def kernel(**inputs) -> np.ndarray:
    out, _ = run_gru(inputs, seq=SEQ, t_blk=T_BLK)
    return out
